# revision 10
# baseline (speedup 1.0000x reference)
"""Trainium2 Bass kernel for nn_BoxHead: 2-layer MLP + class/box heads.

reference:
    h  = relu(feat @ W1 + b1)      # [N, 1024]
    x  = relu(h @ W2 + b2)         # [N, 1024]
    cl = x @ Wc + bc               # [N, 4]
    bx = x @ Wr + br               # [N, 12]
    returns (cl, bx)

Strategy: data-parallel over the proposal dim N (16384) across 8 cores
(2048 rows each). Everything is computed in TRANSPOSED orientation
(hidden dim on SBUF partitions): the host passes X^T per shard, so all
weights DMA in natural layout, biases become per-partition activation
biases, and the two heads fuse into one [1024, 16] matmul. Matmuls run
in bf16 (1 cycle/row on the PE) with fp32 PSUM accumulation.
"""

import sys

sys.path.insert(0, "/opt/trn_rl_repo")

import numpy as np
import ml_dtypes

import concourse.mybir as mybir
from concourse import bacc
from concourse.tile import TileContext
from concourse.bass_utils import run_bass_kernel_spmd

BF16 = mybir.dt.bfloat16
F32 = mybir.dt.float32
NPBF16 = ml_dtypes.bfloat16

N = 16384
D_IN = 12544
HID = 1024
NOUT = 16  # 4 class logits + 12 box regression
NCORES = 8
NS = N // NCORES  # 2048 rows per core

KT1 = D_IN // 128  # 98 k-tiles for layer 1
KT2 = HID // 128  # 8 k-tiles for layer 2 / heads
MT = HID // 128  # 8 m-tiles of the hidden dim
RBLK = 512  # row-block (matmul moving free dim)
NBLK = NS // RBLK  # 4 row blocks per core

_CACHE = {}


def _build():
    nc = bacc.Bacc(None, target_bir_lowering=False, debug=False)

    xt = nc.declare_dram_parameter("xt", [D_IN, NS], BF16, isOutput=False)
    w1 = nc.declare_dram_parameter("w1", [D_IN, HID], BF16, isOutput=False)
    w2 = nc.declare_dram_parameter("w2", [HID, HID], BF16, isOutput=False)
    wh = nc.declare_dram_parameter("wh", [HID, NOUT], BF16, isOutput=False)
    b1 = nc.declare_dram_parameter("b1", [HID], F32, isOutput=False)
    b2 = nc.declare_dram_parameter("b2", [HID], F32, isOutput=False)
    bh = nc.declare_dram_parameter("bh", [NOUT], F32, isOutput=False)
    out = nc.declare_dram_parameter("out", [NOUT, NS], F32, isOutput=True)

    Relu = mybir.ActivationFunctionType.Relu
    Copy = mybir.ActivationFunctionType.Copy

    with TileContext(nc) as tc:
        with (
            tc.tile_pool(name="consts", bufs=1) as consts,
            tc.tile_pool(name="xt_pool", bufs=3) as xt_pool,
            tc.tile_pool(name="w1_pool", bufs=3) as w1_pool,
            tc.tile_pool(name="h_pool", bufs=2 * MT) as h_pool,
            tc.tile_pool(name="x2_pool", bufs=MT + 2) as x2_pool,
            tc.tile_pool(name="o_pool", bufs=2) as o_pool,
            tc.tile_pool(name="psum", bufs=8, space="PSUM") as psum_pool,
        ):
            # resident weights / biases
            w2_t = consts.tile([128, KT2, HID], BF16)
            nc.gpsimd.dma_start(w2_t[:], w2.rearrange("(kt p) n -> p kt n", p=128))
            wh_t = consts.tile([128, KT2, NOUT], BF16)
            nc.gpsimd.dma_start(wh_t[:], wh.rearrange("(kt p) n -> p kt n", p=128))
            b1_t = consts.tile([128, MT], F32)
            nc.gpsimd.dma_start(b1_t[:], b1.rearrange("(m p) -> p m", p=128))
            b2_t = consts.tile([128, MT], F32)
            nc.gpsimd.dma_start(b2_t[:], b2.rearrange("(m p) -> p m", p=128))
            bh_t = consts.tile([NOUT, 1], F32)
            nc.gpsimd.dma_start(bh_t[:], bh[:, None])

            # PE warm-up: keep the HAM activity window busy while the first
            # k-tiles stream in, so the real matmul stream starts at 2.4 GHz.
            wu_sb = consts.tile([128, 128], BF16)
            nc.vector.memset(wu_sb[:], 0.0)
            wu_ps = psum_pool.tile([128, RBLK], F32, name="ps")
            for _ in range(30):
                nc.tensor.matmul(
                    wu_ps[:, :128], lhsT=wu_sb[:], rhs=wu_sb[:],
                    start=True, stop=True,
                )

            # k-tile DMA batch sizes: small at first (fast pipeline fill),
            # then large (amortize per-DMA fixed cost). Sums to KT1 = 98.
            KBATCH = [1, 1, 2, 2, 4, 4] + [7] * 12
            assert sum(KBATCH) == KT1
            KBMAX = max(KBATCH)
            xt_r = xt.rearrange("(kt p) r -> p kt r", p=128)
            w1_r = w1.rearrange("(kt p) n -> p kt n", p=128)

            for blk in range(NBLK):
                rows = slice(blk * RBLK, (blk + 1) * RBLK)

                # ---- layer 1: H^T[:, rows] = relu(W1^T @ X^T + b1) ----
                psums = [psum_pool.tile([128, RBLK], F32, name="ps") for _ in range(MT)]
                k = 0
                for kb in KBATCH:
                    xt_t = xt_pool.tile([128, KBMAX, RBLK], BF16, name="xt_t")
                    nc.sync.dma_start(
                        xt_t[:, :kb, :], xt_r[:, k : k + kb, rows]
                    )
                    w1_t = w1_pool.tile([128, KBMAX, HID], BF16, name="w1_t")
                    nc.sync.dma_start(w1_t[:, :kb, :], w1_r[:, k : k + kb, :])
                    for kk in range(kb):
                        for m in range(MT):
                            nc.tensor.matmul(
                                psums[m][:],
                                lhsT=w1_t[:, kk, m * 128 : (m + 1) * 128],
                                rhs=xt_t[:, kk, :],
                                start=(k + kk == 0),
                                stop=(k + kk == KT1 - 1),
                            )
                    k += kb
                h_tiles = []
                for m in range(MT):
                    h_t = h_pool.tile([128, RBLK], BF16, name="h_t")
                    nc.scalar.activation(
                        h_t[:], psums[m][:], Relu, bias=b1_t[:, m : m + 1]
                    )
                    h_tiles.append(h_t)

                # ---- layer 2: X2^T[:, rows] = relu(W2^T @ H^T + b2) ----
                psums2 = [psum_pool.tile([128, RBLK], F32, name="ps") for _ in range(MT)]
                for k in range(KT2):
                    for m in range(MT):
                        nc.tensor.matmul(
                            psums2[m][:],
                            lhsT=w2_t[:, k, m * 128 : (m + 1) * 128],
                            rhs=h_tiles[k][:],
                            start=(k == 0),
                            stop=(k == KT2 - 1),
                        )
                x2_tiles = []
                for m in range(MT):
                    x2_t = x2_pool.tile([128, RBLK], BF16, name="x2_t")
                    nc.scalar.activation(
                        x2_t[:], psums2[m][:], Relu, bias=b2_t[:, m : m + 1]
                    )
                    x2_tiles.append(x2_t)

                # ---- heads: out[:, rows] = Wh^T @ X2^T + bh ----
                psum_h = psum_pool.tile([128, RBLK], F32, name="ps")
                for k in range(KT2):
                    nc.tensor.matmul(
                        psum_h[:NOUT, :],
                        lhsT=wh_t[:, k, :],
                        rhs=x2_tiles[k][:],
                        start=(k == 0),
                        stop=(k == KT2 - 1),
                    )
                o_t = o_pool.tile([NOUT, RBLK], F32, name="o_t")
                nc.scalar.activation(
                    o_t[:], psum_h[:NOUT, :],
                    mybir.ActivationFunctionType.Identity, bias=bh_t[:],
                )
                nc.sync.dma_start(out[:, rows], o_t[:])

    nc.compile()
    return nc


def _prep_inputs(inputs):
    fv = np.asarray(inputs["feature_vectors"], dtype=np.float32)
    w1 = np.asarray(inputs["W1"], dtype=np.float32).astype(NPBF16)
    w2 = np.asarray(inputs["W2"], dtype=np.float32).astype(NPBF16)
    wh = np.concatenate(
        [np.asarray(inputs["Wc"], dtype=np.float32),
         np.asarray(inputs["Wr"], dtype=np.float32)],
        axis=1,
    ).astype(NPBF16)
    b1 = np.asarray(inputs["b1"], dtype=np.float32)
    b2 = np.asarray(inputs["b2"], dtype=np.float32)
    bhd = np.concatenate(
        [np.asarray(inputs["bc"], dtype=np.float32),
         np.asarray(inputs["br"], dtype=np.float32)]
    )
    in_maps = []
    for c in range(NCORES):
        xt = fv[c * NS : (c + 1) * NS, :].T.astype(NPBF16)  # [D_IN, NS] contig
        in_maps.append(
            {"xt": xt, "w1": w1, "w2": w2, "wh": wh, "b1": b1, "b2": b2, "bh": bhd}
        )
    return in_maps


def _run(inputs, trace=False):
    if "nc" not in _CACHE:
        _CACHE["nc"] = _build()
    nc = _CACHE["nc"]
    in_maps = _prep_inputs(inputs)
    res = run_bass_kernel_spmd(nc, in_maps, list(range(NCORES)), trace=trace)
    outT = np.concatenate([res.results[c]["out"] for c in range(NCORES)], axis=1)
    full = np.ascontiguousarray(outT.T, dtype=np.float32)  # [N, 16]
    class_logits = full[:, :4].copy()
    box_pred = full[:, 4:].copy()
    return (class_logits, box_pred), res


def kernel(**inputs):
    (class_logits, box_pred), _ = _run(inputs, trace=False)
    return class_logits, box_pred


# revision 11
# speedup vs baseline: 1.0177x; 1.0177x over previous
"""Trainium2 Bass kernel for nn_BoxHead: 2-layer MLP + class/box heads.

reference:
    h  = relu(feat @ W1 + b1)      # [N, 1024]
    x  = relu(h @ W2 + b2)         # [N, 1024]
    cl = x @ Wc + bc               # [N, 4]
    bx = x @ Wr + br               # [N, 12]
    returns (cl, bx)

Strategy: data-parallel over the proposal dim N (16384) across 8 cores
(2048 rows each). Everything is computed in TRANSPOSED orientation
(hidden dim on SBUF partitions): the host passes X^T per shard, so all
weights DMA in natural layout, biases become per-partition activation
biases, and the two heads fuse into one [1024, 16] matmul. Matmuls run
in bf16 (1 cycle/row on the PE) with fp32 PSUM accumulation.
"""

import sys

sys.path.insert(0, "/opt/trn_rl_repo")

import numpy as np
import ml_dtypes

import concourse.mybir as mybir
from concourse import bacc
from concourse.tile import TileContext
from concourse.bass_utils import run_bass_kernel_spmd

BF16 = mybir.dt.bfloat16
F32 = mybir.dt.float32
NPBF16 = ml_dtypes.bfloat16

N = 16384
D_IN = 12544
HID = 1024
NOUT = 16  # 4 class logits + 12 box regression
NCORES = 8
NS = N // NCORES  # 2048 rows per core

KT1 = D_IN // 128  # 98 k-tiles for layer 1
KT2 = HID // 128  # 8 k-tiles for layer 2 / heads
MT = HID // 128  # 8 m-tiles of the hidden dim
RBLK = 512  # row-block (matmul moving free dim)
NBLK = NS // RBLK  # 4 row blocks per core

_CACHE = {}


def _build():
    nc = bacc.Bacc(None, target_bir_lowering=False, debug=False)

    # xt is pre-tiled on the host: xt[k, b, p, r] = X[b*RBLK + r, k*128 + p],
    # so each [128, RBLK] matmul tile is one contiguous 128 KB HBM read.
    xt = nc.declare_dram_parameter("xt", [KT1, NBLK, 128, RBLK], BF16, isOutput=False)
    w1 = nc.declare_dram_parameter("w1", [D_IN, HID], BF16, isOutput=False)
    w2 = nc.declare_dram_parameter("w2", [HID, HID], BF16, isOutput=False)
    wh = nc.declare_dram_parameter("wh", [HID, NOUT], BF16, isOutput=False)
    b1 = nc.declare_dram_parameter("b1", [HID], F32, isOutput=False)
    b2 = nc.declare_dram_parameter("b2", [HID], F32, isOutput=False)
    bh = nc.declare_dram_parameter("bh", [NOUT], F32, isOutput=False)
    out = nc.declare_dram_parameter("out", [NOUT, NS], F32, isOutput=True)

    Relu = mybir.ActivationFunctionType.Relu
    Copy = mybir.ActivationFunctionType.Copy

    with TileContext(nc) as tc:
        with (
            tc.tile_pool(name="consts", bufs=1) as consts,
            tc.tile_pool(name="xt_pool", bufs=12) as xt_pool,
            tc.tile_pool(name="w1_pool", bufs=12) as w1_pool,
            tc.tile_pool(name="h_pool", bufs=2 * MT) as h_pool,
            tc.tile_pool(name="x2_pool", bufs=MT + 2) as x2_pool,
            tc.tile_pool(name="o_pool", bufs=2) as o_pool,
            tc.tile_pool(name="psum", bufs=8, space="PSUM") as psum_pool,
        ):
            # resident weights / biases
            w2_t = consts.tile([128, KT2, HID], BF16)
            nc.gpsimd.dma_start(w2_t[:], w2.rearrange("(kt p) n -> p kt n", p=128))
            wh_t = consts.tile([128, KT2, NOUT], BF16)
            nc.gpsimd.dma_start(wh_t[:], wh.rearrange("(kt p) n -> p kt n", p=128))
            b1_t = consts.tile([128, MT], F32)
            nc.gpsimd.dma_start(b1_t[:], b1.rearrange("(m p) -> p m", p=128))
            b2_t = consts.tile([128, MT], F32)
            nc.gpsimd.dma_start(b2_t[:], b2.rearrange("(m p) -> p m", p=128))
            bh_t = consts.tile([NOUT, 1], F32)
            nc.gpsimd.dma_start(bh_t[:], bh[:, None])

            # PE warm-up: keep the HAM activity window busy while the first
            # k-tiles stream in, so the real matmul stream starts at 2.4 GHz.
            wu_sb = consts.tile([128, 128], BF16)
            nc.vector.memset(wu_sb[:], 0.0)
            wu_ps = psum_pool.tile([128, RBLK], F32, name="ps")
            for _ in range(30):
                nc.tensor.matmul(
                    wu_ps[:, :128], lhsT=wu_sb[:], rhs=wu_sb[:],
                    start=True, stop=True,
                )

            for blk in range(NBLK):
                rows = slice(blk * RBLK, (blk + 1) * RBLK)

                # ---- layer 1: H^T[:, rows] = relu(W1^T @ X^T + b1) ----
                psums = [psum_pool.tile([128, RBLK], F32, name="ps") for _ in range(MT)]
                for k in range(KT1):
                    xt_t = xt_pool.tile([128, RBLK], BF16, name="xt_t")
                    nc.sync.dma_start(xt_t[:], xt[k, blk])
                    w1_t = w1_pool.tile([128, HID], BF16, name="w1_t")
                    nc.sync.dma_start(w1_t[:], w1[k * 128 : (k + 1) * 128, :])
                    for m in range(MT):
                        nc.tensor.matmul(
                            psums[m][:],
                            lhsT=w1_t[:, m * 128 : (m + 1) * 128],
                            rhs=xt_t[:],
                            start=(k == 0),
                            stop=(k == KT1 - 1),
                        )
                h_tiles = []
                for m in range(MT):
                    h_t = h_pool.tile([128, RBLK], BF16, name="h_t")
                    nc.scalar.activation(
                        h_t[:], psums[m][:], Relu, bias=b1_t[:, m : m + 1]
                    )
                    h_tiles.append(h_t)

                # ---- layer 2: X2^T[:, rows] = relu(W2^T @ H^T + b2) ----
                psums2 = [psum_pool.tile([128, RBLK], F32, name="ps") for _ in range(MT)]
                for k in range(KT2):
                    for m in range(MT):
                        nc.tensor.matmul(
                            psums2[m][:],
                            lhsT=w2_t[:, k, m * 128 : (m + 1) * 128],
                            rhs=h_tiles[k][:],
                            start=(k == 0),
                            stop=(k == KT2 - 1),
                        )
                x2_tiles = []
                for m in range(MT):
                    x2_t = x2_pool.tile([128, RBLK], BF16, name="x2_t")
                    nc.scalar.activation(
                        x2_t[:], psums2[m][:], Relu, bias=b2_t[:, m : m + 1]
                    )
                    x2_tiles.append(x2_t)

                # ---- heads: out[:, rows] = Wh^T @ X2^T + bh ----
                psum_h = psum_pool.tile([128, RBLK], F32, name="ps")
                for k in range(KT2):
                    nc.tensor.matmul(
                        psum_h[:NOUT, :],
                        lhsT=wh_t[:, k, :],
                        rhs=x2_tiles[k][:],
                        start=(k == 0),
                        stop=(k == KT2 - 1),
                    )
                o_t = o_pool.tile([NOUT, RBLK], F32, name="o_t")
                nc.scalar.activation(
                    o_t[:], psum_h[:NOUT, :],
                    mybir.ActivationFunctionType.Identity, bias=bh_t[:],
                )
                nc.sync.dma_start(out[:, rows], o_t[:])

    nc.compile()
    return nc


def _prep_inputs(inputs):
    fv = np.asarray(inputs["feature_vectors"], dtype=np.float32)
    w1 = np.asarray(inputs["W1"], dtype=np.float32).astype(NPBF16)
    w2 = np.asarray(inputs["W2"], dtype=np.float32).astype(NPBF16)
    wh = np.concatenate(
        [np.asarray(inputs["Wc"], dtype=np.float32),
         np.asarray(inputs["Wr"], dtype=np.float32)],
        axis=1,
    ).astype(NPBF16)
    b1 = np.asarray(inputs["b1"], dtype=np.float32)
    b2 = np.asarray(inputs["b2"], dtype=np.float32)
    bhd = np.concatenate(
        [np.asarray(inputs["bc"], dtype=np.float32),
         np.asarray(inputs["br"], dtype=np.float32)]
    )
    in_maps = []
    for c in range(NCORES):
        shard = fv[c * NS : (c + 1) * NS, :]  # [NS, D_IN]
        # pre-tiled transposed layout: [KT1, NBLK, 128, RBLK]
        xt = np.ascontiguousarray(
            shard.reshape(NBLK, RBLK, KT1, 128).transpose(2, 0, 3, 1)
        ).astype(NPBF16)
        in_maps.append(
            {"xt": xt, "w1": w1, "w2": w2, "wh": wh, "b1": b1, "b2": b2, "bh": bhd}
        )
    return in_maps


def _run(inputs, trace=False):
    if "nc" not in _CACHE:
        _CACHE["nc"] = _build()
    nc = _CACHE["nc"]
    in_maps = _prep_inputs(inputs)
    res = run_bass_kernel_spmd(nc, in_maps, list(range(NCORES)), trace=trace)
    outT = np.concatenate([res.results[c]["out"] for c in range(NCORES)], axis=1)
    full = np.ascontiguousarray(outT.T, dtype=np.float32)  # [N, 16]
    class_logits = full[:, :4].copy()
    box_pred = full[:, 4:].copy()
    return (class_logits, box_pred), res


def kernel(**inputs):
    (class_logits, box_pred), _ = _run(inputs, trace=False)
    return class_logits, box_pred


# revision 12
# speedup vs baseline: 1.0244x; 1.0066x over previous
"""Trainium2 Bass kernel for nn_BoxHead: 2-layer MLP + class/box heads.

reference:
    h  = relu(feat @ W1 + b1)      # [N, 1024]
    x  = relu(h @ W2 + b2)         # [N, 1024]
    cl = x @ Wc + bc               # [N, 4]
    bx = x @ Wr + br               # [N, 12]
    returns (cl, bx)

Strategy: data-parallel over the proposal dim N (16384) across 8 cores
(2048 rows each). Everything is computed in TRANSPOSED orientation
(hidden dim on SBUF partitions): the host passes X^T per shard, so all
weights DMA in natural layout, biases become per-partition activation
biases, and the two heads fuse into one [1024, 16] matmul. Matmuls run
in bf16 (1 cycle/row on the PE) with fp32 PSUM accumulation.
"""

import sys

sys.path.insert(0, "/opt/trn_rl_repo")

import numpy as np
import ml_dtypes

import concourse.mybir as mybir
from concourse import bacc
from concourse.tile import TileContext
from concourse.bass_utils import run_bass_kernel_spmd

BF16 = mybir.dt.bfloat16
F32 = mybir.dt.float32
NPBF16 = ml_dtypes.bfloat16

N = 16384
D_IN = 12544
HID = 1024
NOUT = 16  # 4 class logits + 12 box regression
NCORES = 8
NS = N // NCORES  # 2048 rows per core

KT1 = D_IN // 128  # 98 k-tiles for layer 1
KT2 = HID // 128  # 8 k-tiles for layer 2 / heads
MT = HID // 128  # 8 m-tiles of the hidden dim
RBLK = 512  # row-block (matmul moving free dim)
NBLK = NS // RBLK  # 4 row blocks per core

_CACHE = {}


def _build():
    nc = bacc.Bacc(None, target_bir_lowering=False, debug=False)

    # xt is pre-tiled on the host: xt[k, b, p, r] = X[b*RBLK + r, k*128 + p],
    # so each [128, RBLK] matmul tile is one contiguous 128 KB HBM read.
    xt = nc.declare_dram_parameter("xt", [KT1, NBLK, 128, RBLK], BF16, isOutput=False)
    w1 = nc.declare_dram_parameter("w1", [D_IN, HID], BF16, isOutput=False)
    w2 = nc.declare_dram_parameter("w2", [HID, HID], BF16, isOutput=False)
    wh = nc.declare_dram_parameter("wh", [HID, NOUT], BF16, isOutput=False)
    b1 = nc.declare_dram_parameter("b1", [HID], F32, isOutput=False)
    b2 = nc.declare_dram_parameter("b2", [HID], F32, isOutput=False)
    bh = nc.declare_dram_parameter("bh", [NOUT], F32, isOutput=False)
    out = nc.declare_dram_parameter("out", [NOUT, NS], F32, isOutput=True)

    Relu = mybir.ActivationFunctionType.Relu
    Copy = mybir.ActivationFunctionType.Copy

    with TileContext(nc) as tc:
        with (
            tc.tile_pool(name="consts", bufs=1) as consts,
            tc.tile_pool(name="xt_pool", bufs=12) as xt_pool,
            tc.tile_pool(name="w1_pool", bufs=12) as w1_pool,
            tc.tile_pool(name="h_pool", bufs=2 * MT) as h_pool,
            tc.tile_pool(name="x2_pool", bufs=MT + 2) as x2_pool,
            tc.tile_pool(name="o_pool", bufs=2) as o_pool,
            tc.tile_pool(name="psum", bufs=8, space="PSUM") as psum_pool,
        ):
            # resident weights / biases
            w2_t = consts.tile([128, KT2, HID], BF16)
            nc.gpsimd.dma_start(w2_t[:], w2.rearrange("(kt p) n -> p kt n", p=128))
            wh_t = consts.tile([128, KT2, NOUT], BF16)
            nc.gpsimd.dma_start(wh_t[:], wh.rearrange("(kt p) n -> p kt n", p=128))
            b1_t = consts.tile([128, MT], F32)
            nc.gpsimd.dma_start(b1_t[:], b1.rearrange("(m p) -> p m", p=128))
            b2_t = consts.tile([128, MT], F32)
            nc.gpsimd.dma_start(b2_t[:], b2.rearrange("(m p) -> p m", p=128))
            bh_t = consts.tile([NOUT, 1], F32)
            nc.gpsimd.dma_start(bh_t[:], bh[:, None])

            # PE warm-up: keep the HAM activity window busy while the first
            # k-tiles stream in, so the real matmul stream starts at 2.4 GHz.
            wu_sb = consts.tile([128, 128], BF16)
            nc.vector.memset(wu_sb[:], 0.0)
            wu_ps = psum_pool.tile([128, RBLK], F32, name="ps")
            for _ in range(30):
                nc.tensor.matmul(
                    wu_ps[:, :128], lhsT=wu_sb[:], rhs=wu_sb[:],
                    start=True, stop=True,
                )

            for blk in range(NBLK):
                rows = slice(blk * RBLK, (blk + 1) * RBLK)

                # ---- layer 1: H^T[:, rows] = relu(W1^T @ X^T + b1) ----
                psums = [psum_pool.tile([128, RBLK], F32, name="ps") for _ in range(MT)]
                for k in range(KT1):
                    xt_t = xt_pool.tile([128, RBLK], BF16, name="xt_t")
                    nc.sync.dma_start(xt_t[:], xt[k, blk])
                    w1_t = w1_pool.tile([128, HID], BF16, name="w1_t")
                    nc.sync.dma_start(w1_t[:], w1[k * 128 : (k + 1) * 128, :])
                    for m in range(MT):
                        nc.tensor.matmul(
                            psums[m][:],
                            lhsT=w1_t[:, m * 128 : (m + 1) * 128],
                            rhs=xt_t[:],
                            start=(k == 0),
                            stop=(k == KT1 - 1),
                        )
                h_tiles = []
                for m in range(MT):
                    h_t = h_pool.tile([128, RBLK], BF16, name="h_t")
                    if m % 2 == 0:
                        nc.scalar.activation(
                            h_t[:], psums[m][:], Relu, bias=b1_t[:, m : m + 1]
                        )
                    else:
                        nc.vector.tensor_scalar(
                            h_t[:], psums[m][:], b1_t[:, m : m + 1], 0.0,
                            mybir.AluOpType.add, mybir.AluOpType.max,
                        )
                    h_tiles.append(h_t)

                # ---- layer 2: X2^T[:, rows] = relu(W2^T @ H^T + b2) ----
                psums2 = [psum_pool.tile([128, RBLK], F32, name="ps") for _ in range(MT)]
                for k in range(KT2):
                    for m in range(MT):
                        nc.tensor.matmul(
                            psums2[m][:],
                            lhsT=w2_t[:, k, m * 128 : (m + 1) * 128],
                            rhs=h_tiles[k][:],
                            start=(k == 0),
                            stop=(k == KT2 - 1),
                        )
                x2_tiles = []
                for m in range(MT):
                    x2_t = x2_pool.tile([128, RBLK], BF16, name="x2_t")
                    if m % 2 == 0:
                        nc.scalar.activation(
                            x2_t[:], psums2[m][:], Relu, bias=b2_t[:, m : m + 1]
                        )
                    else:
                        nc.vector.tensor_scalar(
                            x2_t[:], psums2[m][:], b2_t[:, m : m + 1], 0.0,
                            mybir.AluOpType.add, mybir.AluOpType.max,
                        )
                    x2_tiles.append(x2_t)

                # ---- heads: out[:, rows] = Wh^T @ X2^T + bh ----
                psum_h = psum_pool.tile([128, RBLK], F32, name="ps")
                for k in range(KT2):
                    nc.tensor.matmul(
                        psum_h[:NOUT, :],
                        lhsT=wh_t[:, k, :],
                        rhs=x2_tiles[k][:],
                        start=(k == 0),
                        stop=(k == KT2 - 1),
                    )
                o_t = o_pool.tile([NOUT, RBLK], F32, name="o_t")
                nc.scalar.activation(
                    o_t[:], psum_h[:NOUT, :],
                    mybir.ActivationFunctionType.Identity, bias=bh_t[:],
                )
                nc.sync.dma_start(out[:, rows], o_t[:])

    nc.compile()
    return nc


def _prep_inputs(inputs):
    fv = np.asarray(inputs["feature_vectors"], dtype=np.float32)
    w1 = np.asarray(inputs["W1"], dtype=np.float32).astype(NPBF16)
    w2 = np.asarray(inputs["W2"], dtype=np.float32).astype(NPBF16)
    wh = np.concatenate(
        [np.asarray(inputs["Wc"], dtype=np.float32),
         np.asarray(inputs["Wr"], dtype=np.float32)],
        axis=1,
    ).astype(NPBF16)
    b1 = np.asarray(inputs["b1"], dtype=np.float32)
    b2 = np.asarray(inputs["b2"], dtype=np.float32)
    bhd = np.concatenate(
        [np.asarray(inputs["bc"], dtype=np.float32),
         np.asarray(inputs["br"], dtype=np.float32)]
    )
    in_maps = []
    for c in range(NCORES):
        shard = fv[c * NS : (c + 1) * NS, :]  # [NS, D_IN]
        # pre-tiled transposed layout: [KT1, NBLK, 128, RBLK]
        xt = np.ascontiguousarray(
            shard.reshape(NBLK, RBLK, KT1, 128).transpose(2, 0, 3, 1)
        ).astype(NPBF16)
        in_maps.append(
            {"xt": xt, "w1": w1, "w2": w2, "wh": wh, "b1": b1, "b2": b2, "bh": bhd}
        )
    return in_maps


def _run(inputs, trace=False):
    if "nc" not in _CACHE:
        _CACHE["nc"] = _build()
    nc = _CACHE["nc"]
    in_maps = _prep_inputs(inputs)
    res = run_bass_kernel_spmd(nc, in_maps, list(range(NCORES)), trace=trace)
    outT = np.concatenate([res.results[c]["out"] for c in range(NCORES)], axis=1)
    full = np.ascontiguousarray(outT.T, dtype=np.float32)  # [N, 16]
    class_logits = full[:, :4].copy()
    box_pred = full[:, 4:].copy()
    return (class_logits, box_pred), res


def kernel(**inputs):
    (class_logits, box_pred), _ = _run(inputs, trace=False)
    return class_logits, box_pred


# revision 13
# speedup vs baseline: 1.0294x; 1.0048x over previous
"""Trainium2 Bass kernel for nn_BoxHead: 2-layer MLP + class/box heads.

reference:
    h  = relu(feat @ W1 + b1)      # [N, 1024]
    x  = relu(h @ W2 + b2)         # [N, 1024]
    cl = x @ Wc + bc               # [N, 4]
    bx = x @ Wr + br               # [N, 12]
    returns (cl, bx)

Strategy: data-parallel over the proposal dim N (16384) across 8 cores
(2048 rows each). Everything is computed in TRANSPOSED orientation
(hidden dim on SBUF partitions): the host passes X^T per shard, so all
weights DMA in natural layout, biases become per-partition activation
biases, and the two heads fuse into one [1024, 16] matmul. Matmuls run
in bf16 (1 cycle/row on the PE) with fp32 PSUM accumulation.
"""

import sys

sys.path.insert(0, "/opt/trn_rl_repo")

import numpy as np
import ml_dtypes

import concourse.mybir as mybir
from concourse import bacc
from concourse.tile import TileContext
from concourse.bass_utils import run_bass_kernel_spmd

BF16 = mybir.dt.bfloat16
F32 = mybir.dt.float32
NPBF16 = ml_dtypes.bfloat16

N = 16384
D_IN = 12544
HID = 1024
NOUT = 16  # 4 class logits + 12 box regression
NCORES = 8
NS = N // NCORES  # 2048 rows per core

KT1 = D_IN // 128  # 98 k-tiles for layer 1
KT2 = HID // 128  # 8 k-tiles for layer 2 / heads
MT = HID // 128  # 8 m-tiles of the hidden dim
RBLK = 512  # row-block (matmul moving free dim)
NBLK = NS // RBLK  # 4 row blocks per core

_CACHE = {}


def _build():
    nc = bacc.Bacc(None, target_bir_lowering=False, debug=False)

    # xt is pre-tiled on the host: xt[k, b, p, r] = X[b*RBLK + r, k*128 + p],
    # so each [128, RBLK] matmul tile is one contiguous 128 KB HBM read.
    xt = nc.declare_dram_parameter("xt", [KT1, NBLK, 128, RBLK], BF16, isOutput=False)
    w1 = nc.declare_dram_parameter("w1", [D_IN, HID], BF16, isOutput=False)
    w2 = nc.declare_dram_parameter("w2", [HID, HID], BF16, isOutput=False)
    wh = nc.declare_dram_parameter("wh", [HID, NOUT], BF16, isOutput=False)
    b1 = nc.declare_dram_parameter("b1", [HID], F32, isOutput=False)
    b2 = nc.declare_dram_parameter("b2", [HID], F32, isOutput=False)
    bh = nc.declare_dram_parameter("bh", [NOUT], F32, isOutput=False)
    out = nc.declare_dram_parameter("out", [NOUT, NS], F32, isOutput=True)

    Relu = mybir.ActivationFunctionType.Relu
    Copy = mybir.ActivationFunctionType.Copy

    with TileContext(nc) as tc:
        with (
            tc.tile_pool(name="consts", bufs=1) as consts,
            tc.tile_pool(name="xt_pool", bufs=12) as xt_pool,
            tc.tile_pool(name="w1_pool", bufs=12) as w1_pool,
            tc.tile_pool(name="h_pool", bufs=2 * MT) as h_pool,
            tc.tile_pool(name="x2_pool", bufs=MT + 2) as x2_pool,
            tc.tile_pool(name="o_pool", bufs=2) as o_pool,
            tc.tile_pool(name="psum", bufs=8, space="PSUM") as psum_pool,
        ):
            # resident weights / biases
            w2_t = consts.tile([128, KT2, HID], BF16)
            nc.gpsimd.dma_start(w2_t[:], w2.rearrange("(kt p) n -> p kt n", p=128))
            wh_t = consts.tile([128, KT2, NOUT], BF16)
            nc.gpsimd.dma_start(wh_t[:], wh.rearrange("(kt p) n -> p kt n", p=128))
            b1_t = consts.tile([128, MT], F32)
            nc.gpsimd.dma_start(b1_t[:], b1.rearrange("(m p) -> p m", p=128))
            b2_t = consts.tile([128, MT], F32)
            nc.gpsimd.dma_start(b2_t[:], b2.rearrange("(m p) -> p m", p=128))
            bh_t = consts.tile([NOUT, 1], F32)
            nc.gpsimd.dma_start(bh_t[:], bh[:, None])

            # PE warm-up: keep the HAM activity window busy while the first
            # k-tiles stream in, so the real matmul stream starts at 2.4 GHz.
            wu_sb = consts.tile([128, 128], BF16)
            nc.vector.memset(wu_sb[:], 0.0)
            wu_ps = psum_pool.tile([128, RBLK], F32, name="ps")
            for _ in range(30):
                nc.tensor.matmul(
                    wu_ps[:, :128], lhsT=wu_sb[:], rhs=wu_sb[:],
                    start=True, stop=True,
                )

            for blk in range(NBLK):
                rows = slice(blk * RBLK, (blk + 1) * RBLK)

                # ---- layer 1: H^T[:, rows] = relu(W1^T @ X^T + b1) ----
                psums = [psum_pool.tile([128, RBLK], F32, name="ps") for _ in range(MT)]
                for k in range(KT1):
                    xt_t = xt_pool.tile([128, RBLK], BF16, name="xt_t")
                    nc.sync.dma_start(xt_t[:], xt[k, blk])
                    w1_t = w1_pool.tile([128, HID], BF16, name="w1_t")
                    nc.scalar.dma_start(w1_t[:], w1[k * 128 : (k + 1) * 128, :])
                    for m in range(MT):
                        nc.tensor.matmul(
                            psums[m][:],
                            lhsT=w1_t[:, m * 128 : (m + 1) * 128],
                            rhs=xt_t[:],
                            start=(k == 0),
                            stop=(k == KT1 - 1),
                        )
                h_tiles = []
                for m in range(MT):
                    h_t = h_pool.tile([128, RBLK], BF16, name="h_t")
                    if m % 2 == 0:
                        nc.scalar.activation(
                            h_t[:], psums[m][:], Relu, bias=b1_t[:, m : m + 1]
                        )
                    else:
                        nc.vector.tensor_scalar(
                            h_t[:], psums[m][:], b1_t[:, m : m + 1], 0.0,
                            mybir.AluOpType.add, mybir.AluOpType.max,
                        )
                    h_tiles.append(h_t)

                # ---- layer 2: X2^T[:, rows] = relu(W2^T @ H^T + b2) ----
                psums2 = [psum_pool.tile([128, RBLK], F32, name="ps") for _ in range(MT)]
                for k in range(KT2):
                    for m in range(MT):
                        nc.tensor.matmul(
                            psums2[m][:],
                            lhsT=w2_t[:, k, m * 128 : (m + 1) * 128],
                            rhs=h_tiles[k][:],
                            start=(k == 0),
                            stop=(k == KT2 - 1),
                        )
                x2_tiles = []
                for m in range(MT):
                    x2_t = x2_pool.tile([128, RBLK], BF16, name="x2_t")
                    if m % 2 == 0:
                        nc.scalar.activation(
                            x2_t[:], psums2[m][:], Relu, bias=b2_t[:, m : m + 1]
                        )
                    else:
                        nc.vector.tensor_scalar(
                            x2_t[:], psums2[m][:], b2_t[:, m : m + 1], 0.0,
                            mybir.AluOpType.add, mybir.AluOpType.max,
                        )
                    x2_tiles.append(x2_t)

                # ---- heads: out[:, rows] = Wh^T @ X2^T + bh ----
                psum_h = psum_pool.tile([128, RBLK], F32, name="ps")
                for k in range(KT2):
                    nc.tensor.matmul(
                        psum_h[:NOUT, :],
                        lhsT=wh_t[:, k, :],
                        rhs=x2_tiles[k][:],
                        start=(k == 0),
                        stop=(k == KT2 - 1),
                    )
                o_t = o_pool.tile([NOUT, RBLK], F32, name="o_t")
                nc.scalar.activation(
                    o_t[:], psum_h[:NOUT, :],
                    mybir.ActivationFunctionType.Identity, bias=bh_t[:],
                )
                nc.sync.dma_start(out[:, rows], o_t[:])

    nc.compile()
    return nc


def _prep_inputs(inputs):
    fv = np.asarray(inputs["feature_vectors"], dtype=np.float32)
    w1 = np.asarray(inputs["W1"], dtype=np.float32).astype(NPBF16)
    w2 = np.asarray(inputs["W2"], dtype=np.float32).astype(NPBF16)
    wh = np.concatenate(
        [np.asarray(inputs["Wc"], dtype=np.float32),
         np.asarray(inputs["Wr"], dtype=np.float32)],
        axis=1,
    ).astype(NPBF16)
    b1 = np.asarray(inputs["b1"], dtype=np.float32)
    b2 = np.asarray(inputs["b2"], dtype=np.float32)
    bhd = np.concatenate(
        [np.asarray(inputs["bc"], dtype=np.float32),
         np.asarray(inputs["br"], dtype=np.float32)]
    )
    in_maps = []
    for c in range(NCORES):
        shard = fv[c * NS : (c + 1) * NS, :]  # [NS, D_IN]
        # pre-tiled transposed layout: [KT1, NBLK, 128, RBLK]
        xt = np.ascontiguousarray(
            shard.reshape(NBLK, RBLK, KT1, 128).transpose(2, 0, 3, 1)
        ).astype(NPBF16)
        in_maps.append(
            {"xt": xt, "w1": w1, "w2": w2, "wh": wh, "b1": b1, "b2": b2, "bh": bhd}
        )
    return in_maps


def _run(inputs, trace=False):
    if "nc" not in _CACHE:
        _CACHE["nc"] = _build()
    nc = _CACHE["nc"]
    in_maps = _prep_inputs(inputs)
    res = run_bass_kernel_spmd(nc, in_maps, list(range(NCORES)), trace=trace)
    outT = np.concatenate([res.results[c]["out"] for c in range(NCORES)], axis=1)
    full = np.ascontiguousarray(outT.T, dtype=np.float32)  # [N, 16]
    class_logits = full[:, :4].copy()
    box_pred = full[:, 4:].copy()
    return (class_logits, box_pred), res


def kernel(**inputs):
    (class_logits, box_pred), _ = _run(inputs, trace=False)
    return class_logits, box_pred
